# revision 52
# baseline (speedup 1.0000x reference)
"""Trainium2 Bass kernel for nn_BinaryClassifier (FFT-frame-mean + 3-layer MLP).

Math: the reference computes sigmoid(relu(relu(Re(mean_f FFT(x_f)) @ W1.T +
b1) @ W2.T + b2) @ W3.T + b3). Both the frame-mean and the FFT are linear and
only the real part survives, so
    Re(mean_f FFT(x_f)) = (sum_f x_f) @ (C / 31),  C[n,k] = cos(2*pi*n*k/N)
and layer 1 folds to  relu( (sum_f x_f) @ W1c + b1 )  with W1c = C @ W1.T / 31
precomputed on host in float64.

Sharding: pure data parallel; 1024 batch rows / 8 cores = 128 rows = one SBUF
partition dim per core. Weights replicated.

The whole-input stream is the roofline: the per-core DMA fabric tops out at
~435 GB/s (measured; ntff metadata dma_ddr_bandwidth agrees), so the only
lever that matters is shipping fewer bytes and keeping everything else off
the critical path. This build:

- Ships x as fp16 (host cast): 16.25 MB/core instead of 32.5 -> ~37 us
  stream. fp16 quantization of N(0,1) data adds ~2e-4 relative error.
- Ships x PRE-TRANSPOSED on host to the [feature, batch] layout layer 1
  consumes (column index = q*31*1024 + f*1024 + k'*128 + b for feature halves
  q), so the 31-frame sum lands directly in matmul-rhs orientation: the 16
  [128,128] PE transposes, the f32r rounding dance and the PSUM merge of the
  old layout all disappear.
- Splits features into two halves streamed sequentially: half 0's layer-1
  matmuls (16 of the 32) run while half 1 is still streaming, so the
  post-stream tail is only the final adds + 16 matmuls + the tiny MLP.
- Sums frames on DVE with fp16 in-place adds (16-bit DVE ops run at 2x; a
  [128,1024] add is ~0.5 us, 62 of them fit easily inside the stream).
  Optionally (BASS_PE_FRAMES>0) the first pe_n frames of each half go to PE
  as identity-stationary accumulating matmuls into PSUM instead, with a
  DVE merge at the end of the half -- kept as a fallback should DVE 2x not
  hold; at 0 the merge degenerates to nothing and PE only does the MLP.
- W1c ships as fp16 (not bf16): same bytes, 3 more mantissa bits, and layer 1
  runs single-pass fp16 x fp16.
- Weights go on the gpsimd DMA queue so the two main queues carry only x.

Hardware constraints navigated (from the previous fp32 build, all verified
on axon trn2): one sync-wait per instruction (Bacc splits multi-waits;
dummy pre-join matmuls/activations keep hot-path PE/ACT ops at one data
dependency), and a cold ACT sigmoid table load (~1.3us) is preloaded during
the stream by a dummy sigmoid at kernel start.
"""

import os
from contextlib import ExitStack

import numpy as np

import concourse.bacc as bacc
import concourse.bass as bass
import concourse.tile as tile
from concourse import mybir
from concourse.bass_utils import run_bass_kernel_spmd

FRAMES = 31
FFT_LEN = 2048
B = 1024
NCORES = 8
BS = B // NCORES  # 128
H1 = 256
H2 = 256
P = 128
KCH = FFT_LEN // P  # 16 feature chunks of 128
NQ = 2  # feature halves streamed sequentially
KH = KCH // NQ  # 8 chunks per half
HW = KH * P  # 1024 features per half

F16 = mybir.dt.float16
F32 = mybir.dt.float32

# column layout of the packed fp32 constants tensor wpk [128, NW]: biases only
B10 = 0  # 2 cols
B20 = B10 + 2  # 2 cols
B30 = B20 + 2  # 1 col
NW = B30 + 1
# fp16 constants tensor wph [128, NWH]: W1c, W2T, W3T
W2TH = KCH * H1  # 2 chunks x 256
W3TH = W2TH + 2 * H2  # 2 cols
NWH = W3TH + 2

# frames of each half handled by PE identity-matmul accumulation (rest: DVE
# in-place adds). 0 = pure DVE, 31 = pure PE.
PE_FRAMES = int(os.environ.get("BASS_PE_FRAMES", "14"))
# frames per x DMA: an int (uniform), or 0 for the tapered plan (small groups
# at the start of each half so consumption ramps before a big group lands, and
# at the end so the final adds aren't gated on a 4-frame transfer)
G = int(os.environ.get("BASS_DMA_GROUP", "4"))
TAPER = (1, 2, 4, 4, 4, 4, 4, 4, 4)  # start-taper only; sums to 31


def group_plan(g: int) -> list[int]:
    if g == 0:
        assert sum(TAPER) == FRAMES
        return list(TAPER)
    return [min(g, FRAMES - f0) for f0 in range(0, FRAMES, g)]
# DVE accumulator: "f16" (in-place), "f32" (in-place fp32), or "pp"
# (fp16 ping-pong: out is a different buffer from both inputs, the pattern
# that lets the DVE 16-bit 2x perf mode engage)
ACC = os.environ.get("BASS_ACC", "f16")


def build_nc(
    pe_n: int = PE_FRAMES, g: int = G, acc: str = ACC, dbg: bool = False
) -> bass.Bass:
    nc = bacc.Bacc("TRN2", debug=False)

    x_h = nc.dram_tensor("xt", [P, FRAMES * FFT_LEN], F16, kind="ExternalInput")
    wpk_h = nc.dram_tensor("wpk", [P, NW], F32, kind="ExternalInput")
    wph_h = nc.dram_tensor("wph", [P, NWH], F16, kind="ExternalInput")
    wid_h = nc.dram_tensor("wid", [P, P], F16, kind="ExternalInput")
    out_h = nc.dram_tensor("out", [1, BS], F32, kind="ExternalOutput")
    if dbg:
        dbg_h = nc.dram_tensor("dbg", [P, FFT_LEN], F16, kind="ExternalOutput")
        dbgh1_h = nc.dram_tensor("dbg_h1", [P, H1], F32, kind="ExternalOutput")

    x = x_h.ap()  # [128, 2*31*1024]: feat-on-partitions, (half, frame, chunk, batch)

    with ExitStack() as ctx:
        tc = ctx.enter_context(tile.TileContext(nc))
        singles = ctx.enter_context(tc.tile_pool(name="singles", bufs=1))
        state = ctx.enter_context(tc.tile_pool(name="state", bufs=1))
        frames_pool = ctx.enter_context(tc.tile_pool(name="frames", bufs=8))
        ph1 = ctx.enter_context(tc.tile_pool(name="ph1", bufs=1, space="PSUM"))
        pwork = ctx.enter_context(
            tc.tile_pool(name="pwork", bufs=(1 if pe_n > 0 else 2), space="PSUM")
        )
        if pe_n > 0:
            psum_s = ctx.enter_context(
                tc.tile_pool(name="psum_s", bufs=1, space="PSUM")
            )

        plan = group_plan(g)
        NG = len(plan)
        gmax = max(plan)

        # Weight placement keeps gpsimd empty (its preamble/drain/barrier
        # slices were the longest in the epilogue) and never blocks a DMA
        # queue behind a weight wait:
        # - sync queue: wid (fp16 identity, 32KB) + wpk (biases, 2.5KB) first
        #   -- they land ~instantly -- then the even x groups, out at the end.
        # - scalar queue: odd x groups; after its 3rd group issue, the wph DMA
        #   (1.2MB, needed by layer 1 at ~30us) and the ACT pre-joins (the scr
        #   copy observes wpk's sem once; the dummy sigmoid pulls the ~1.3us
        #   ACT table load into the stream). By then wpk has landed, so these
        #   never stall the queue's remaining x issues.
        wpk = singles.tile([P, NW], F32)
        wph = singles.tile([P, NWH], F16)
        wid = singles.tile([P, P], F16)
        scr = state.tile([1, 1], F32, tag="scr")
        # weights on the gpsimd (SWDGE) queue: the two HW queues carry only x
        nc.gpsimd.dma_start(out=wid, in_=wid_h.ap())
        nc.gpsimd.dma_start(out=wpk, in_=wpk_h.ap())
        nc.gpsimd.dma_start(out=wph, in_=wph_h.ap())

        def load_group(q, gi, f0, nf):
            t = frames_pool.tile([P, gmax * HW], F16, tag="xg")
            gn = q * NG + gi
            eng = nc.sync if gn % 2 == 0 else nc.scalar
            c0 = q * FRAMES * HW + f0 * HW
            eng.dma_start(out=t[:, : nf * HW], in_=x[:, c0 : c0 + nf * HW])
            if gn == 5:
                # ACT pre-joins, emitted mid-stream so they never stall the
                # scalar queue's early x issues: the scr copy observes wpk's
                # sem once; the dummy sigmoid pulls the ~1.3us ACT table load
                # into the stream.
                nc.scalar.activation(
                    scr, wpk[0:1, 0:1], mybir.ActivationFunctionType.Copy,
                    bias=0.0, scale=1.0,
                )
                nc.scalar.activation(
                    scr, wpk[0:1, 0:1], mybir.ActivationFunctionType.Sigmoid,
                    bias=0.0, scale=1.0,
                )
            return t

        def w1c(k, m):
            c0 = k * H1 + m * P
            return wph[:, c0 : c0 + P]

        def w2t(k, m):
            c0 = W2TH + k * H2 + m * P
            return wph[:, c0 : c0 + P]

        # PE pre-join: observe the wid DMA once (the first identity matmul
        # then has only its frame-data wait)
        identh = wid
        if pe_n > 0:
            dummy_ps = pwork.tile([1, 1], F32, tag="pw")
            nc.tensor.matmul(
                dummy_ps, lhsT=wid[:, 0:1], rhs=wid[:, 0:1], start=True, stop=True
            )
        # frames handled by PE, spread evenly through the stream (and covering
        # the tail so the serial DVE chain doesn't gate the merge)
        pe_set = {
            f for f in range(FRAMES)
            if ((f + 1) * pe_n) // FRAMES > (f * pe_n) // FRAMES
        }

        # One PSUM tile (= bank) per m half: two interleaved accumulation
        # groups in a single bank corrupt each other -- a start=True on one
        # group resets the whole bank (measured: m=0 lost its first chunk).
        h1p = [
            ph1.tile([P, P], F32, tag=f"h1p{m}", name=f"h1p{m}") for m in range(2)
        ]
        h1_sb = state.tile([P, H1], F16, tag="h1_sb")

        acc_dt = F32 if acc == "f32" else F16
        for q in range(NQ):
            # ---- frame sum for half q ----
            if pe_n > 0:
                s_ps = psum_s.tile([P, HW], F32, tag="sps")
            s_dve = None
            s_alt = None
            first_dve = True
            n_pe = 0
            f0 = 0
            for gi, nf in enumerate(plan):
                xt = load_group(q, gi, f0, nf)
                for j in range(nf):
                    f = f0 + j
                    sl = xt[:, j * HW : (j + 1) * HW]
                    if f in pe_set:
                        # N=512 chunks: one accumulation chain per PSUM bank
                        for cc in range(HW // 512):
                            nc.tensor.matmul(
                                s_ps[:, cc * 512 : (cc + 1) * 512],
                                lhsT=identh,
                                rhs=sl[:, cc * 512 : (cc + 1) * 512],
                                start=(n_pe == 0),
                                stop=(n_pe == pe_n - 1),
                            )
                        n_pe += 1
                        continue
                    if first_dve:
                        s_dve = state.tile([P, HW], acc_dt, tag=f"sdve{q}")
                        if acc == "pp":
                            s_alt = state.tile([P, HW], acc_dt, tag=f"salt{q}")
                        nc.vector.tensor_copy(s_dve, sl)
                        first_dve = False
                    elif acc == "pp":
                        nc.vector.tensor_add(s_alt, s_dve, sl)
                        s_dve, s_alt = s_alt, s_dve
                    else:
                        nc.vector.tensor_add(s_dve, s_dve, sl)
                f0 += nf
            if q == 0:
                # PE pre-join on wph (landed mid-stream) before the first L1
                # matmul so L1 ops keep a single data-dependency wait
                dummy_w = pwork.tile([1, 1], F32, tag="pw")
                nc.tensor.matmul(
                    dummy_w, lhsT=wph[:, 0:1], rhs=wph[:, 0:1],
                    start=True, stop=True,
                )
            if pe_n > 0 and s_dve is not None:
                # merge PE partial into the fp16 sum (also the L1 rhs); two
                # chunks so the first L1 matmuls can overlap the second chunk
                sT = state.tile([P, HW], F16, tag=f"st{q}")
                for c in range(2):
                    cs = slice(c * HW // 2, (c + 1) * HW // 2)
                    nc.vector.tensor_add(sT[:, cs], s_dve[:, cs], s_ps[:, cs])
            elif pe_n > 0:
                sT = state.tile([P, HW], F16, tag=f"st{q}")
                nc.vector.tensor_copy(sT, s_ps)
            elif acc in ("f16", "pp"):
                sT = s_dve
            else:
                sT = state.tile([P, HW], F16, tag=f"st{q}")
                nc.vector.tensor_copy(sT, s_dve)
            if dbg:
                nc.gpsimd.dma_start(out=dbg_h.ap()[:, q * HW : (q + 1) * HW], in_=sT)

            # ---- layer 1 for half q (accumulates across halves) ----
            for k in range(KH):
                kg = q * KH + k
                for m in range(2):
                    nc.tensor.matmul(
                        h1p[m],
                        lhsT=w1c(kg, m),
                        rhs=sT[:, k * P : (k + 1) * P],
                        start=(kg == 0),
                        stop=(kg == KCH - 1),
                    )

        for m in range(2):
            nc.scalar.activation(
                h1_sb[:, m * P : (m + 1) * P],
                h1p[m],
                mybir.ActivationFunctionType.Relu,
                bias=wpk[:, B10 + m : B10 + m + 1],
                scale=1.0,
            )

        if dbg:
            dbg_h1sb = state.tile([P, H1], F32, tag="dbg_h1sb")
            for m in range(2):
                nc.vector.tensor_copy(dbg_h1sb[:, m * P : (m + 1) * P], h1p[m])
            nc.gpsimd.dma_start(out=dbgh1_h.ap(), in_=dbg_h1sb)

        # ---- layer 2 (fp16 weights + fp16 rhs: single-pass PE) ----
        h2_sb = state.tile([P, H2], F16, tag="h2_sb")
        for m in range(2):
            h2p = pwork.tile([P, P], F32, tag="pw2")
            for k in range(2):
                nc.tensor.matmul(
                    h2p,
                    lhsT=w2t(k, m),
                    rhs=h1_sb[:, k * P : (k + 1) * P],
                    start=(k == 0),
                    stop=(k == 1),
                )
            nc.scalar.activation(
                h2_sb[:, m * P : (m + 1) * P],
                h2p,
                mybir.ActivationFunctionType.Relu,
                bias=wpk[:, B20 + m : B20 + m + 1],
                scale=1.0,
            )

        # ---- layer 3 + sigmoid ----
        op = pwork.tile([1, P], F32, tag="pw_o")
        for k in range(2):
            nc.tensor.matmul(
                op,
                lhsT=wph[:, W3TH + k : W3TH + k + 1],
                rhs=h2_sb[:, k * P : (k + 1) * P],
                start=(k == 0),
                stop=(k == 1),
            )
        o_sb = state.tile([1, BS], F32, tag="o_sb")
        nc.scalar.activation(
            o_sb,
            op,
            mybir.ActivationFunctionType.Sigmoid,
            bias=wpk[0:1, B30 : B30 + 1],
            scale=1.0,
        )
        nc.sync.dma_start(out=out_h.ap(), in_=o_sb)

    nc.compile()
    return nc


_NC_CACHE: dict = {}


def _get_nc(pe_n: int = PE_FRAMES, g: int = G, acc: str = ACC) -> bass.Bass:
    key = (pe_n, g, acc)
    if key not in _NC_CACHE:
        _NC_CACHE[key] = build_nc(pe_n, g, acc)
    return _NC_CACHE[key]


_HOST_CACHE: dict = {}


def _host_weights(W1, b1, W2, b2, W3, b3):
    key = (W1.__array_interface__["data"][0], W1.shape)
    if key in _HOST_CACHE:
        return _HOST_CACHE[key]

    n = np.arange(FFT_LEN)
    ang = (2.0 * np.pi / FFT_LEN) * ((n[:, None] * n[None, :]) % FFT_LEN)
    C = np.cos(ang)  # float64 [2048, 2048]
    W1c = (C @ W1.astype(np.float64).T / FRAMES).astype(np.float32)  # [2048, 256]
    W2T = W2.astype(np.float32).T  # [256, 256]
    W3T = W3.astype(np.float32).T.reshape(H2)  # [256]

    wpk = np.zeros((P, NW), dtype=np.float32)
    for m in range(2):
        wpk[:, B10 + m] = b1.astype(np.float32)[m * P : (m + 1) * P]
        wpk[:, B20 + m] = b2.astype(np.float32)[m * P : (m + 1) * P]
    wpk[:, B30] = np.float32(b3.reshape(-1)[0])

    wph = np.zeros((P, NWH), dtype=np.float16)
    for k in range(KCH):
        wph[:, k * H1 : (k + 1) * H1] = W1c[k * P : (k + 1) * P, :].astype(
            np.float16
        )
    for k in range(2):
        wph[:, W2TH + k * H2 : W2TH + (k + 1) * H2] = W2T[
            k * P : (k + 1) * P, :
        ].astype(np.float16)
    for k in range(2):
        wph[:, W3TH + k] = W3T[k * P : (k + 1) * P].astype(np.float16)
    wid = np.eye(P, dtype=np.float16)

    pack = {"wpk": wpk, "wph": wph, "wid": wid}
    _HOST_CACHE[key] = pack
    return pack


def _shard_x(x: np.ndarray) -> list[np.ndarray]:
    """Per-core fp16 tensors in [feature, (half, frame, chunk, batch)] layout:
    xt[p, q*31*1024 + f*1024 + k'*128 + b] = x[b, f*2048 + (q*8+k')*128 + p].
    """
    xh = x.astype(np.float16)
    shards = []
    for c in range(NCORES):
        xc = xh[c * BS : (c + 1) * BS]  # [128b, 63488]
        v = xc.reshape(BS, FRAMES, NQ, KH, P)  # b f q k p
        v = v.transpose(4, 2, 1, 3, 0)  # p q f k b
        shards.append(np.ascontiguousarray(v.reshape(P, FRAMES * FFT_LEN)))
    return shards


def kernel(x, W1, b1, W2, b2, W3, b3, _trace=False, _pe_n=None, _g=None, _acc=None):
    pe_n = PE_FRAMES if _pe_n is None else _pe_n
    g = G if _g is None else _g
    acc = ACC if _acc is None else _acc
    x = np.asarray(x, dtype=np.float32)
    pack = _host_weights(
        np.asarray(W1), np.asarray(b1), np.asarray(W2),
        np.asarray(b2), np.asarray(W3), np.asarray(b3),
    )
    in_maps = [{"xt": xt, **pack} for xt in _shard_x(x)]
    nc = _get_nc(pe_n, g, acc)
    res = run_bass_kernel_spmd(
        nc, in_maps, core_ids=list(range(NCORES)), trace=_trace
    )
    out = np.concatenate([res.results[c]["out"][0] for c in range(NCORES)])
    out = out.reshape(B, 1).astype(np.float32)
    if _trace:
        return out, res
    return out


# revision 54
# speedup vs baseline: 1.0616x; 1.0616x over previous
"""Trainium2 Bass kernel for nn_BinaryClassifier (FFT-frame-mean + 3-layer MLP).

Math: the reference computes sigmoid(relu(relu(Re(mean_f FFT(x_f)) @ W1.T +
b1) @ W2.T + b2) @ W3.T + b3). Both the frame-mean and the FFT are linear and
only the real part survives, so
    Re(mean_f FFT(x_f)) = (sum_f x_f) @ (C / 31),  C[n,k] = cos(2*pi*n*k/N)
and layer 1 folds to  relu( (sum_f x_f) @ W1c + b1 )  with W1c = C @ W1.T / 31
precomputed on host in float64.

Sharding: pure data parallel; 1024 batch rows / 8 cores = 128 rows = one SBUF
partition dim per core. Weights replicated.

The whole-input stream is the roofline: the per-core DMA fabric tops out at
~435 GB/s (measured; ntff metadata dma_ddr_bandwidth agrees), so the only
lever that matters is shipping fewer bytes and keeping everything else off
the critical path. This build:

- Ships x as fp16 (host cast): 16.25 MB/core instead of 32.5 -> ~37 us
  stream. fp16 quantization of N(0,1) data adds ~2e-4 relative error.
- Ships x PRE-TRANSPOSED on host to the [feature, batch] layout layer 1
  consumes (column index = q*31*1024 + f*1024 + k'*128 + b for feature halves
  q), so the 31-frame sum lands directly in matmul-rhs orientation: the 16
  [128,128] PE transposes, the f32r rounding dance and the PSUM merge of the
  old layout all disappear.
- Splits features into two halves streamed sequentially: half 0's layer-1
  matmuls (16 of the 32) run while half 1 is still streaming, so the
  post-stream tail is only the final adds + 16 matmuls + the tiny MLP.
- Sums frames on DVE with fp16 in-place adds (16-bit DVE ops run at 2x; a
  [128,1024] add is ~0.5 us, 62 of them fit easily inside the stream).
  Optionally (BASS_PE_FRAMES>0) the first pe_n frames of each half go to PE
  as identity-stationary accumulating matmuls into PSUM instead, with a
  DVE merge at the end of the half -- kept as a fallback should DVE 2x not
  hold; at 0 the merge degenerates to nothing and PE only does the MLP.
- W1c ships as fp16 (not bf16): same bytes, 3 more mantissa bits, and layer 1
  runs single-pass fp16 x fp16.
- Weights go on the gpsimd DMA queue so the two main queues carry only x.

Hardware constraints navigated (from the previous fp32 build, all verified
on axon trn2): one sync-wait per instruction (Bacc splits multi-waits;
dummy pre-join matmuls/activations keep hot-path PE/ACT ops at one data
dependency), and a cold ACT sigmoid table load (~1.3us) is preloaded during
the stream by a dummy sigmoid at kernel start.
"""

import os
from contextlib import ExitStack

import numpy as np

import concourse.bacc as bacc
import concourse.bass as bass
import concourse.tile as tile
from concourse import mybir
from concourse.bass_utils import run_bass_kernel_spmd

FRAMES = 31
FFT_LEN = 2048
B = 1024
NCORES = 8
BS = B // NCORES  # 128
H1 = 256
H2 = 256
P = 128
KCH = FFT_LEN // P  # 16 feature chunks of 128
NQ = 2  # feature halves streamed sequentially
KH = KCH // NQ  # 8 chunks per half
HW = KH * P  # 1024 features per half

F16 = mybir.dt.float16
F32 = mybir.dt.float32

# column layout of the packed fp32 constants tensor wpk [128, NW]: biases only
B10 = 0  # 2 cols
B20 = B10 + 2  # 2 cols
B30 = B20 + 2  # 1 col
NW = B30 + 1
# fp16 constants tensor wph [128, NWH]: W1c, W2T, W3T
W2TH = KCH * H1  # 2 chunks x 256
W3TH = W2TH + 2 * H2  # 2 cols
NWH = W3TH + 2

# frames of each half handled by PE identity-matmul accumulation (rest: DVE
# in-place adds). 0 = pure DVE, 31 = pure PE.
PE_FRAMES = int(os.environ.get("BASS_PE_FRAMES", "14"))
# frames per x DMA: an int (uniform), or 0 for the tapered plan (small groups
# at the start of each half so consumption ramps before a big group lands, and
# at the end so the final adds aren't gated on a 4-frame transfer)
G = int(os.environ.get("BASS_DMA_GROUP", "4"))
TAPER = (1, 2, 4, 4, 4, 4, 4, 4, 4)  # start-taper only; sums to 31


def group_plan(g: int) -> list[int]:
    if g == 0:
        assert sum(TAPER) == FRAMES
        return list(TAPER)
    return [min(g, FRAMES - f0) for f0 in range(0, FRAMES, g)]
# DVE accumulator: "f16" (in-place), "f32" (in-place fp32), or "pp"
# (fp16 ping-pong: out is a different buffer from both inputs, the pattern
# that lets the DVE 16-bit 2x perf mode engage)
ACC = os.environ.get("BASS_ACC", "f16")


def build_nc(
    pe_n: int = PE_FRAMES, g: int = G, acc: str = ACC, dbg: bool = False
) -> bass.Bass:
    nc = bacc.Bacc("TRN2", debug=False)

    x_h = nc.dram_tensor("xt", [P, FRAMES * FFT_LEN], F16, kind="ExternalInput")
    wpk_h = nc.dram_tensor("wpk", [P, NW], F32, kind="ExternalInput")
    wph_h = nc.dram_tensor("wph", [P, NWH], F16, kind="ExternalInput")
    wid_h = nc.dram_tensor("wid", [P, P], F16, kind="ExternalInput")
    out_h = nc.dram_tensor("out", [1, BS], F32, kind="ExternalOutput")
    if dbg:
        dbg_h = nc.dram_tensor("dbg", [P, FFT_LEN], F16, kind="ExternalOutput")
        dbgh1_h = nc.dram_tensor("dbg_h1", [P, H1], F32, kind="ExternalOutput")

    x = x_h.ap()  # [128, 2*31*1024]: feat-on-partitions, (half, frame, chunk, batch)

    with ExitStack() as ctx:
        tc = ctx.enter_context(tile.TileContext(nc))
        singles = ctx.enter_context(tc.tile_pool(name="singles", bufs=1))
        state = ctx.enter_context(tc.tile_pool(name="state", bufs=1))
        frames_pool = ctx.enter_context(tc.tile_pool(name="frames", bufs=8))
        ph1 = ctx.enter_context(tc.tile_pool(name="ph1", bufs=1, space="PSUM"))
        pwork = ctx.enter_context(
            tc.tile_pool(name="pwork", bufs=(1 if pe_n > 0 else 2), space="PSUM")
        )
        if pe_n > 0:
            psum_s = ctx.enter_context(
                tc.tile_pool(name="psum_s", bufs=1, space="PSUM")
            )

        plan = group_plan(g)
        NG = len(plan)
        gmax = max(plan)

        # Weight placement keeps gpsimd empty (its preamble/drain/barrier
        # slices were the longest in the epilogue) and never blocks a DMA
        # queue behind a weight wait:
        # - sync queue: wid (fp16 identity, 32KB) + wpk (biases, 2.5KB) first
        #   -- they land ~instantly -- then the even x groups, out at the end.
        # - scalar queue: odd x groups; after its 3rd group issue, the wph DMA
        #   (1.2MB, needed by layer 1 at ~30us) and the ACT pre-joins (the scr
        #   copy observes wpk's sem once; the dummy sigmoid pulls the ~1.3us
        #   ACT table load into the stream). By then wpk has landed, so these
        #   never stall the queue's remaining x issues.
        wpk = singles.tile([P, NW], F32)
        wph = singles.tile([P, NWH], F16)
        wid = singles.tile([P, P], F16)
        scr = state.tile([1, 1], F32, tag="scr")
        # weights on the gpsimd (SWDGE) queue: the two HW queues carry only x
        nc.gpsimd.dma_start(out=wid, in_=wid_h.ap())
        nc.gpsimd.dma_start(out=wpk, in_=wpk_h.ap())
        nc.gpsimd.dma_start(out=wph, in_=wph_h.ap())

        def load_group(q, gi, f0, nf):
            t = frames_pool.tile([P, gmax * HW], F16, tag="xg")
            gn = q * NG + gi
            eng = nc.sync if gn % 2 == 0 else nc.scalar
            c0 = q * FRAMES * HW + f0 * HW
            eng.dma_start(out=t[:, : nf * HW], in_=x[:, c0 : c0 + nf * HW])
            return t

        def act_prejoins():
            # ACT pre-joins, emitted after the scalar queue's last x issue so
            # they never delay a group: the scr copy observes wpk's sem once;
            # the dummy sigmoid pulls the ~1.3us ACT table load into the
            # stream tail (still well before the first relu).
            nc.scalar.activation(
                scr, wpk[0:1, 0:1], mybir.ActivationFunctionType.Copy,
                bias=0.0, scale=1.0,
            )
            nc.scalar.activation(
                scr, wpk[0:1, 0:1], mybir.ActivationFunctionType.Sigmoid,
                bias=0.0, scale=1.0,
            )

        def w1c(k, m):
            c0 = k * H1 + m * P
            return wph[:, c0 : c0 + P]

        def w2t(k, m):
            c0 = W2TH + k * H2 + m * P
            return wph[:, c0 : c0 + P]

        # PE pre-join: observe the wid DMA once (the first identity matmul
        # then has only its frame-data wait)
        identh = wid
        if pe_n > 0:
            dummy_ps = pwork.tile([1, 1], F32, tag="pw")
            nc.tensor.matmul(
                dummy_ps, lhsT=wid[:, 0:1], rhs=wid[:, 0:1], start=True, stop=True
            )
        # frames handled by PE, spread evenly through the stream (and covering
        # the tail so the serial DVE chain doesn't gate the merge)
        pe_set = {
            f for f in range(FRAMES)
            if ((f + 1) * pe_n) // FRAMES > (f * pe_n) // FRAMES
        }

        # One PSUM tile (= bank) per m half: two interleaved accumulation
        # groups in a single bank corrupt each other -- a start=True on one
        # group resets the whole bank (measured: m=0 lost its first chunk).
        h1p = [
            ph1.tile([P, P], F32, tag=f"h1p{m}", name=f"h1p{m}") for m in range(2)
        ]
        h1_sb = state.tile([P, H1], F16, tag="h1_sb")

        acc_dt = F32 if acc == "f32" else F16
        for q in range(NQ):
            # ---- frame sum for half q ----
            if pe_n > 0:
                s_ps = psum_s.tile([P, HW], F32, tag="sps")
            s_dve = None
            s_alt = None
            first_dve = True
            n_pe = 0
            f0 = 0
            for gi, nf in enumerate(plan):
                xt = load_group(q, gi, f0, nf)
                for j in range(nf):
                    f = f0 + j
                    sl = xt[:, j * HW : (j + 1) * HW]
                    if f in pe_set:
                        # N=512 chunks: one accumulation chain per PSUM bank
                        for cc in range(HW // 512):
                            nc.tensor.matmul(
                                s_ps[:, cc * 512 : (cc + 1) * 512],
                                lhsT=identh,
                                rhs=sl[:, cc * 512 : (cc + 1) * 512],
                                start=(n_pe == 0),
                                stop=(n_pe == pe_n - 1),
                            )
                        n_pe += 1
                        continue
                    if first_dve:
                        s_dve = state.tile([P, HW], acc_dt, tag=f"sdve{q}")
                        if acc == "pp":
                            s_alt = state.tile([P, HW], acc_dt, tag=f"salt{q}")
                        nc.vector.tensor_copy(s_dve, sl)
                        first_dve = False
                    elif acc == "pp":
                        nc.vector.tensor_add(s_alt, s_dve, sl)
                        s_dve, s_alt = s_alt, s_dve
                    else:
                        nc.vector.tensor_add(s_dve, s_dve, sl)
                f0 += nf
            if q == 0:
                # PE pre-join on wph (landed mid-stream) before the first L1
                # matmul so L1 ops keep a single data-dependency wait
                dummy_w = pwork.tile([1, 1], F32, tag="pw")
                nc.tensor.matmul(
                    dummy_w, lhsT=wph[:, 0:1], rhs=wph[:, 0:1],
                    start=True, stop=True,
                )
            if pe_n > 0 and s_dve is not None:
                # merge PE partial into the fp16 sum (also the L1 rhs); two
                # chunks so the first L1 matmuls can overlap the second chunk
                sT = state.tile([P, HW], F16, tag=f"st{q}")
                for c in range(2):
                    cs = slice(c * HW // 2, (c + 1) * HW // 2)
                    nc.vector.tensor_add(sT[:, cs], s_dve[:, cs], s_ps[:, cs])
            elif pe_n > 0:
                sT = state.tile([P, HW], F16, tag=f"st{q}")
                nc.vector.tensor_copy(sT, s_ps)
            elif acc in ("f16", "pp"):
                sT = s_dve
            else:
                sT = state.tile([P, HW], F16, tag=f"st{q}")
                nc.vector.tensor_copy(sT, s_dve)
            if dbg:
                nc.gpsimd.dma_start(out=dbg_h.ap()[:, q * HW : (q + 1) * HW], in_=sT)

            # ---- layer 1 for half q (accumulates across halves) ----
            for k in range(KH):
                kg = q * KH + k
                for m in range(2):
                    nc.tensor.matmul(
                        h1p[m],
                        lhsT=w1c(kg, m),
                        rhs=sT[:, k * P : (k + 1) * P],
                        start=(kg == 0),
                        stop=(kg == KCH - 1),
                    )

        act_prejoins()
        for m in range(2):
            nc.scalar.activation(
                h1_sb[:, m * P : (m + 1) * P],
                h1p[m],
                mybir.ActivationFunctionType.Relu,
                bias=wpk[:, B10 + m : B10 + m + 1],
                scale=1.0,
            )

        if dbg:
            dbg_h1sb = state.tile([P, H1], F32, tag="dbg_h1sb")
            for m in range(2):
                nc.vector.tensor_copy(dbg_h1sb[:, m * P : (m + 1) * P], h1p[m])
            nc.gpsimd.dma_start(out=dbgh1_h.ap(), in_=dbg_h1sb)

        # ---- layer 2 (fp16 weights + fp16 rhs: single-pass PE) ----
        h2_sb = state.tile([P, H2], F16, tag="h2_sb")
        for m in range(2):
            h2p = pwork.tile([P, P], F32, tag="pw2")
            for k in range(2):
                nc.tensor.matmul(
                    h2p,
                    lhsT=w2t(k, m),
                    rhs=h1_sb[:, k * P : (k + 1) * P],
                    start=(k == 0),
                    stop=(k == 1),
                )
            nc.scalar.activation(
                h2_sb[:, m * P : (m + 1) * P],
                h2p,
                mybir.ActivationFunctionType.Relu,
                bias=wpk[:, B20 + m : B20 + m + 1],
                scale=1.0,
            )

        # ---- layer 3 + sigmoid ----
        op = pwork.tile([1, P], F32, tag="pw_o")
        for k in range(2):
            nc.tensor.matmul(
                op,
                lhsT=wph[:, W3TH + k : W3TH + k + 1],
                rhs=h2_sb[:, k * P : (k + 1) * P],
                start=(k == 0),
                stop=(k == 1),
            )
        o_sb = state.tile([1, BS], F32, tag="o_sb")
        nc.scalar.activation(
            o_sb,
            op,
            mybir.ActivationFunctionType.Sigmoid,
            bias=wpk[0:1, B30 : B30 + 1],
            scale=1.0,
        )
        nc.sync.dma_start(out=out_h.ap(), in_=o_sb)

    nc.compile()
    return nc


_NC_CACHE: dict = {}


def _get_nc(pe_n: int = PE_FRAMES, g: int = G, acc: str = ACC) -> bass.Bass:
    key = (pe_n, g, acc)
    if key not in _NC_CACHE:
        _NC_CACHE[key] = build_nc(pe_n, g, acc)
    return _NC_CACHE[key]


_HOST_CACHE: dict = {}


def _host_weights(W1, b1, W2, b2, W3, b3):
    key = (W1.__array_interface__["data"][0], W1.shape)
    if key in _HOST_CACHE:
        return _HOST_CACHE[key]

    n = np.arange(FFT_LEN)
    ang = (2.0 * np.pi / FFT_LEN) * ((n[:, None] * n[None, :]) % FFT_LEN)
    C = np.cos(ang)  # float64 [2048, 2048]
    W1c = (C @ W1.astype(np.float64).T / FRAMES).astype(np.float32)  # [2048, 256]
    W2T = W2.astype(np.float32).T  # [256, 256]
    W3T = W3.astype(np.float32).T.reshape(H2)  # [256]

    wpk = np.zeros((P, NW), dtype=np.float32)
    for m in range(2):
        wpk[:, B10 + m] = b1.astype(np.float32)[m * P : (m + 1) * P]
        wpk[:, B20 + m] = b2.astype(np.float32)[m * P : (m + 1) * P]
    wpk[:, B30] = np.float32(b3.reshape(-1)[0])

    wph = np.zeros((P, NWH), dtype=np.float16)
    for k in range(KCH):
        wph[:, k * H1 : (k + 1) * H1] = W1c[k * P : (k + 1) * P, :].astype(
            np.float16
        )
    for k in range(2):
        wph[:, W2TH + k * H2 : W2TH + (k + 1) * H2] = W2T[
            k * P : (k + 1) * P, :
        ].astype(np.float16)
    for k in range(2):
        wph[:, W3TH + k] = W3T[k * P : (k + 1) * P].astype(np.float16)
    wid = np.eye(P, dtype=np.float16)

    pack = {"wpk": wpk, "wph": wph, "wid": wid}
    _HOST_CACHE[key] = pack
    return pack


def _shard_x(x: np.ndarray) -> list[np.ndarray]:
    """Per-core fp16 tensors in [feature, (half, frame, chunk, batch)] layout:
    xt[p, q*31*1024 + f*1024 + k'*128 + b] = x[b, f*2048 + (q*8+k')*128 + p].
    """
    xh = x.astype(np.float16)
    shards = []
    for c in range(NCORES):
        xc = xh[c * BS : (c + 1) * BS]  # [128b, 63488]
        v = xc.reshape(BS, FRAMES, NQ, KH, P)  # b f q k p
        v = v.transpose(4, 2, 1, 3, 0)  # p q f k b
        shards.append(np.ascontiguousarray(v.reshape(P, FRAMES * FFT_LEN)))
    return shards


def kernel(x, W1, b1, W2, b2, W3, b3, _trace=False, _pe_n=None, _g=None, _acc=None):
    pe_n = PE_FRAMES if _pe_n is None else _pe_n
    g = G if _g is None else _g
    acc = ACC if _acc is None else _acc
    x = np.asarray(x, dtype=np.float32)
    pack = _host_weights(
        np.asarray(W1), np.asarray(b1), np.asarray(W2),
        np.asarray(b2), np.asarray(W3), np.asarray(b3),
    )
    in_maps = [{"xt": xt, **pack} for xt in _shard_x(x)]
    nc = _get_nc(pe_n, g, acc)
    res = run_bass_kernel_spmd(
        nc, in_maps, core_ids=list(range(NCORES)), trace=_trace
    )
    out = np.concatenate([res.results[c]["out"][0] for c in range(NCORES)])
    out = out.reshape(B, 1).astype(np.float32)
    if _trace:
        return out, res
    return out
